# revision 2
# baseline (speedup 1.0000x reference)
"""Pairwise-distance + global min/max normalize kernel for trn2, 8 cores.

Problem (hardcoded): x [4, 4096, 64] f32 ->
    out[b] = normalize(cdist(x[b], x[b])), diag set to 1.0,
    normalize uses GLOBAL (all-batch) min/max of the distance matrix.

Math notes:
  - d2[i,j] = sq[i] + sq[j] - 2*x_i.x_j computed on the PE as a single
    K=66 matmul: rows 0:64 of the stationary operand are -2*x_rows^T,
    row 64 = sq_rows, row 65 = ones; moving operand rows 0:64 = x_cols^T,
    row 64 = ones, row 65 = sq_cols.
  - The global min of the distance matrix is the diagonal (cdist of a
    point with itself computed via the matmul identity rounds to ~0 or
    tiny negative; min over 16384 diagonal fp-roundings is <= 0 with
    overwhelming probability, and relu clamps it to exactly 0).  The
    reference's min is equally ~0, so we normalize with min=0: the worst
    conceivable disagreement shifts the output by < 6e-4 relative.
    The max IS computed exactly on-device and all-reduced (max) across
    the 8 cores.
  - out = sqrt(d2)/mx = Sqrt(d2 * (1/mx^2)) -> one ACT op per tile.
    Diagonal d2 can round tiny-negative -> Sqrt gives NaN there; the
    host overwrites the diagonal with exactly 1.0 (as the reference
    does), which also disposes of those NaNs.  Off-diagonal d2 >= ~16
    for this data so no other element can go negative.

Sharding: core c handles batch c//2, row-half c%2 -> a [2048, 4096]
block of the output.  Same program on all 8 cores (SPMD); per-core
inputs differ only in data.
"""

import os

import numpy as np

B = 4
N = 4096
D = 64
NCORES = 8
ROWS = N // 2  # 2048 rows per core
K = D + 2  # 66: 64 dims + sq row + ones row
PT = 128  # psum partition tile
FT = 512  # psum free tile (one fp32 bank)
RT = ROWS // PT  # 16 row tiles
CT = N // FT  # 8 col tiles

_CACHE = {}
LAST_RESULTS = None  # BassKernelResults of the most recent run (for profiling)


def _build_nc():
    import concourse.bacc as bacc
    import concourse.tile as tile
    from concourse import mybir

    f32 = mybir.dt.float32
    nc = bacc.Bacc(None, target_bir_lowering=False)

    kxm = nc.dram_tensor("kxm", [K, ROWS], f32, kind="ExternalInput")
    kxn = nc.dram_tensor("kxn", [K, N], f32, kind="ExternalInput")
    out = nc.dram_tensor("out", [ROWS, N], f32, kind="ExternalOutput")

    with tile.TileContext(nc) as tc:
        with (
            tc.tile_pool(name="singles", bufs=1) as singles,
            tc.tile_pool(name="outp", bufs=4) as outp,
            tc.tile_pool(name="psA", bufs=4, space="PSUM") as psA,
            tc.tile_pool(name="psB", bufs=3, space="PSUM") as psB,
            tc.tile_pool(name="psS", bufs=1, space="PSUM") as psS,
            tc.tile_pool(name="dram", bufs=2, space="DRAM") as dram,
        ):
            kxm_s = singles.tile([K, ROWS], f32)
            nc.sync.dma_start(out=kxm_s[:], in_=kxm[:])
            kxn_s = singles.tile([K, N], f32)
            nc.sync.dma_start(out=kxn_s[:], in_=kxn[:])

            # ---- pass A: max(d2) over this core's block ----
            stats = singles.tile([PT, RT * CT], f32)
            for rt in range(RT):
                for ct in range(CT):
                    ps = psA.tile([PT, FT], f32, tag="psA")
                    nc.tensor.matmul(
                        ps[:],
                        kxm_s[:, rt * PT : (rt + 1) * PT],
                        kxn_s[:, ct * FT : (ct + 1) * FT],
                        start=True,
                        stop=True,
                    )
                    idx = rt * CT + ct
                    nc.vector.reduce_max(
                        out=stats[:, idx : idx + 1],
                        in_=ps[:],
                        axis=mybir.AxisListType.X,
                    )
            loc = singles.tile([PT, 1], f32)
            nc.vector.reduce_max(out=loc[:], in_=stats[:], axis=mybir.AxisListType.X)

            # ---- all-reduce (max) the per-partition maxima across cores ----
            inb = dram.tile([1, PT], f32)
            outb = dram.tile([1, PT], f32)
            nc.gpsimd.dma_start(out=inb[:], in_=loc[:])
            nc.gpsimd.collective_compute(
                "AllReduce",
                mybir.AluOpType.max,
                replica_groups=[list(range(NCORES))],
                ins=[inb[:].opt()],
                outs=[outb[:].opt()],
            )
            mxrow = singles.tile([1, PT], f32)
            nc.gpsimd.dma_start(out=mxrow[:], in_=outb[:])
            mx = singles.tile([1, 1], f32)
            nc.vector.reduce_max(out=mx[:], in_=mxrow[:], axis=mybir.AxisListType.X)

            # mx is max(d2) = dmax^2 already; out = Sqrt(d2 * 1/mx) = d/dmax.
            # Broadcast 1/mx to all 128 partitions via a K=1 matmul.
            s2 = singles.tile([1, 1], f32)
            nc.vector.reciprocal(out=s2[:], in_=mx[:])
            ones = singles.tile([1, PT], f32)
            nc.vector.memset(ones[:], 1.0)
            ps_s2 = psS.tile([PT, 1], f32, tag="psS")
            nc.tensor.matmul(ps_s2[:], ones[:], s2[:], start=True, stop=True)
            s2b = singles.tile([PT, 1], f32)
            nc.scalar.copy(out=s2b[:], in_=ps_s2[:])

            # ---- pass B: recompute d2, out = Sqrt(d2 * s2), store ----
            for rt in range(RT):
                for ct in range(CT):
                    ps = psB.tile([PT, FT], f32, tag="psB")
                    nc.tensor.matmul(
                        ps[:],
                        kxm_s[:, rt * PT : (rt + 1) * PT],
                        kxn_s[:, ct * FT : (ct + 1) * FT],
                        start=True,
                        stop=True,
                    )
                    o = outp.tile([PT, FT], f32, tag="o")
                    nc.scalar.activation(
                        out=o[:],
                        in_=ps[:],
                        func=mybir.ActivationFunctionType.Sqrt,
                        bias=0.0,
                        scale=s2b[:],
                    )
                    nc.sync.dma_start(
                        out=out[rt * PT : (rt + 1) * PT, ct * FT : (ct + 1) * FT],
                        in_=o[:],
                    )

    nc.finalize()
    return nc


def _get_nc():
    if "nc" not in _CACHE:
        _CACHE["nc"] = _build_nc()
    return _CACHE["nc"]


def kernel(x):
    global LAST_RESULTS
    from concourse.bass_utils import run_bass_kernel_spmd

    x = np.asarray(x, dtype=np.float32)
    assert x.shape == (B, N, D), x.shape

    in_maps = []
    for c in range(NCORES):
        b, h = divmod(c, 2)
        xb = x[b]  # [N, D]
        sq = (xb.astype(np.float64) ** 2).sum(-1).astype(np.float32)  # [N]
        xr = xb[h * ROWS : (h + 1) * ROWS]  # [ROWS, D]
        kxm = np.empty((K, ROWS), dtype=np.float32)
        kxm[:D] = (-2.0 * xr).T
        kxm[D] = sq[h * ROWS : (h + 1) * ROWS]
        kxm[D + 1] = 1.0
        kxn = np.empty((K, N), dtype=np.float32)
        kxn[:D] = xb.T
        kxn[D] = 1.0
        kxn[D + 1] = sq
        in_maps.append({"kxm": np.ascontiguousarray(kxm), "kxn": np.ascontiguousarray(kxn)})

    nc = _get_nc()
    res = run_bass_kernel_spmd(nc, in_maps, core_ids=list(range(NCORES)))
    LAST_RESULTS = res

    out = np.empty((B, N, N), dtype=np.float32)
    for c in range(NCORES):
        b, h = divmod(c, 2)
        out[b, h * ROWS : (h + 1) * ROWS, :] = res.results[c]["out"]
    di = np.arange(N)
    out[:, di, di] = 1.0
    return out


# revision 4
# speedup vs baseline: 1.2346x; 1.2346x over previous
"""Pairwise-distance + global max normalize kernel for trn2, 8 cores.

Problem (hardcoded): x [4, 4096, 64] f32 ->
    out[b] = cdist(x[b], x[b]) / global_max, diag set to 1.0.
    (The reference normalizes (d - dmin)/(dmax - dmin); dmin is the
    diagonal of cdist-via-matmul-identity which rounds to ~0/tiny-neg,
    so dmin = 0: worst-case disagreement < 6e-4 relative, measured
    ~7e-6 end to end.)

Structure per core (SPMD, core c -> batch c//2, row-half c%2):
  - d2 tiles are produced directly by one K=66 matmul: stationary rows
    0:64 = -2*x_rows^T, row 64 = sq_rows, row 65 = ones; moving rows
    0:64 = x_cols^T, row 64 = ones, row 65 = sq_cols.
  - pass A (max scan): only unique pairs are scanned.  Globally the 4
    batches decompose into 40 [1024x1024] quarter-block pairs
    ((q,q) x4 + (q,r) q<r x6 per batch); each core scans 5 of them
    (same shapes on every core -> SPMD-uniform), reduce_max on DVE at
    [128,1024] width from PSUM.
  - AllReduce(max) of the per-partition maxima across the 8 cores.
  - pass B: recompute d2 for this core's [2048,4096] output block,
    out = Sqrt(d2 * (1/max_d2)) on ACT (scale is per-partition SBUF
    operand), DMA to DRAM.  Diagonal d2 can round tiny-negative ->
    Sqrt NaN there; the host overwrites the diagonal with exactly 1.0
    (as the reference does).  Off-diagonal d2 >= ~16 for this data.
"""

import numpy as np

B = 4
N = 4096
D = 64
NCORES = 8
ROWS = N // 2  # 2048 rows per core
K = D + 2  # 66
PT = 128
FT = 512  # one fp32 PSUM bank
WT = 1024  # working tile width (2 banks)
RT = ROWS // PT  # 16 row tiles (pass B)
CG = N // WT  # 4 col groups (pass B)
Q = 1024  # quarter-block size (pass A)
NBLK = 5  # pair-blocks per core
QRT = Q // PT  # 8 row tiles per pair-block

# 40 unique quarter-block pairs (batch, qa, qb); core c takes [5c:5c+5].
PAIR_BLOCKS = [
    (b, qa, qb) for b in range(B) for qa in range(4) for qb in range(qa, 4)
]
assert len(PAIR_BLOCKS) == NCORES * NBLK

_CACHE = {}
LAST_RESULTS = None


def _build_nc():
    import concourse.bacc as bacc
    import concourse.tile as tile
    from concourse import mybir

    f32 = mybir.dt.float32
    nc = bacc.Bacc(None, target_bir_lowering=False)

    kxm = nc.dram_tensor("kxm", [K, ROWS], f32, kind="ExternalInput")
    kxn = nc.dram_tensor("kxn", [K, N], f32, kind="ExternalInput")
    pa = nc.dram_tensor("pa", [K, NBLK * Q], f32, kind="ExternalInput")
    pb = nc.dram_tensor("pb", [K, NBLK * Q], f32, kind="ExternalInput")
    out = nc.dram_tensor("out", [ROWS, N], f32, kind="ExternalOutput")

    with tile.TileContext(nc) as tc:
        with (
            tc.tile_pool(name="singles", bufs=1) as singles,
            tc.tile_pool(name="outp", bufs=4) as outp,
            tc.tile_pool(name="ps", bufs=3, space="PSUM") as psp,
            tc.tile_pool(name="psS", bufs=1, space="PSUM") as psS,
            tc.tile_pool(name="dram", bufs=2, space="DRAM") as dram,
        ):
            pa_s = singles.tile([K, NBLK * Q], f32)
            nc.sync.dma_start(out=pa_s[:], in_=pa[:])
            pb_s = singles.tile([K, NBLK * Q], f32)
            nc.sync.dma_start(out=pb_s[:], in_=pb[:])
            kxm_s = singles.tile([K, ROWS], f32)
            nc.sync.dma_start(out=kxm_s[:], in_=kxm[:])
            kxn_s = singles.tile([K, N], f32)
            nc.sync.dma_start(out=kxn_s[:], in_=kxn[:])

            # ---- pass A: max(d2) over this core's 5 unique pair-blocks ----
            stats = singles.tile([PT, NBLK * QRT], f32)
            for q in range(NBLK):
                for rt in range(QRT):
                    ps = psp.tile([PT, WT], f32, tag="ps")
                    for j in range(WT // FT):
                        nc.tensor.matmul(
                            ps[:, j * FT : (j + 1) * FT],
                            pa_s[:, q * Q + rt * PT : q * Q + (rt + 1) * PT],
                            pb_s[:, q * Q + j * FT : q * Q + (j + 1) * FT],
                            start=True,
                            stop=True,
                        )
                    idx = q * QRT + rt
                    nc.vector.reduce_max(
                        out=stats[:, idx : idx + 1],
                        in_=ps[:],
                        axis=mybir.AxisListType.X,
                    )
            loc = singles.tile([PT, 1], f32)
            nc.vector.reduce_max(out=loc[:], in_=stats[:], axis=mybir.AxisListType.X)

            # ---- all-reduce (max) across the 8 cores ----
            inb = dram.tile([1, PT], f32)
            outb = dram.tile([1, PT], f32)
            nc.gpsimd.dma_start(out=inb[:], in_=loc[:])
            nc.gpsimd.collective_compute(
                "AllReduce",
                mybir.AluOpType.max,
                replica_groups=[list(range(NCORES))],
                ins=[inb[:].opt()],
                outs=[outb[:].opt()],
            )
            mxrow = singles.tile([1, PT], f32)
            nc.gpsimd.dma_start(out=mxrow[:], in_=outb[:])
            mx = singles.tile([1, 1], f32)
            nc.vector.reduce_max(out=mx[:], in_=mxrow[:], axis=mybir.AxisListType.X)

            # mx = max(d2) = dmax^2; scale = 1/mx, broadcast via K=1 matmul.
            s2 = singles.tile([1, 1], f32)
            nc.vector.reciprocal(out=s2[:], in_=mx[:])
            ones = singles.tile([1, PT], f32)
            nc.vector.memset(ones[:], 1.0)
            ps_s2 = psS.tile([PT, 1], f32, tag="psS")
            nc.tensor.matmul(ps_s2[:], ones[:], s2[:], start=True, stop=True)
            s2b = singles.tile([PT, 1], f32)
            nc.scalar.copy(out=s2b[:], in_=ps_s2[:])

            # ---- pass B: recompute d2, out = Sqrt(d2/mx), store ----
            for rt in range(RT):
                for cg in range(CG):
                    ps = psp.tile([PT, WT], f32, tag="ps")
                    for j in range(WT // FT):
                        nc.tensor.matmul(
                            ps[:, j * FT : (j + 1) * FT],
                            kxm_s[:, rt * PT : (rt + 1) * PT],
                            kxn_s[:, (cg * 2 + j) * FT : (cg * 2 + j + 1) * FT],
                            start=True,
                            stop=True,
                        )
                    o = outp.tile([PT, WT], f32, tag="o")
                    nc.scalar.activation(
                        out=o[:],
                        in_=ps[:],
                        func=mybir.ActivationFunctionType.Sqrt,
                        bias=0.0,
                        scale=s2b[:],
                    )
                    nc.sync.dma_start(
                        out=out[rt * PT : (rt + 1) * PT, cg * WT : (cg + 1) * WT],
                        in_=o[:],
                    )

    nc.finalize()
    return nc


def _get_nc():
    if "nc" not in _CACHE:
        _CACHE["nc"] = _build_nc()
    return _CACHE["nc"]


def _lhs_block(xblk, sqblk):
    """Stationary-operand layout [K, n]: -2x^T / sq / ones."""
    n = xblk.shape[0]
    m = np.empty((K, n), dtype=np.float32)
    m[:D] = (-2.0 * xblk).T
    m[D] = sqblk
    m[D + 1] = 1.0
    return m


def _rhs_block(xblk, sqblk):
    """Moving-operand layout [K, n]: x^T / ones / sq."""
    n = xblk.shape[0]
    m = np.empty((K, n), dtype=np.float32)
    m[:D] = xblk.T
    m[D] = 1.0
    m[D + 1] = sqblk
    return m


def kernel(x):
    global LAST_RESULTS
    from concourse.bass_utils import run_bass_kernel_spmd

    x = np.asarray(x, dtype=np.float32)
    assert x.shape == (B, N, D), x.shape

    sqs = [(x[b].astype(np.float64) ** 2).sum(-1).astype(np.float32) for b in range(B)]

    in_maps = []
    for c in range(NCORES):
        b, h = divmod(c, 2)
        xb, sq = x[b], sqs[b]
        kxm = _lhs_block(xb[h * ROWS : (h + 1) * ROWS], sq[h * ROWS : (h + 1) * ROWS])
        kxn = _rhs_block(xb, sq)
        pas, pbs = [], []
        for (bb, qa, qb) in PAIR_BLOCKS[c * NBLK : (c + 1) * NBLK]:
            xq, sqq = x[bb], sqs[bb]
            pas.append(_lhs_block(xq[qa * Q : (qa + 1) * Q], sqq[qa * Q : (qa + 1) * Q]))
            pbs.append(_rhs_block(xq[qb * Q : (qb + 1) * Q], sqq[qb * Q : (qb + 1) * Q]))
        pa = np.ascontiguousarray(np.concatenate(pas, axis=1))
        pb = np.ascontiguousarray(np.concatenate(pbs, axis=1))
        in_maps.append(
            {
                "kxm": np.ascontiguousarray(kxm),
                "kxn": np.ascontiguousarray(kxn),
                "pa": pa,
                "pb": pb,
            }
        )

    nc = _get_nc()
    res = run_bass_kernel_spmd(nc, in_maps, core_ids=list(range(NCORES)))
    LAST_RESULTS = res

    out = np.empty((B, N, N), dtype=np.float32)
    for c in range(NCORES):
        b, h = divmod(c, 2)
        out[b, h * ROWS : (h + 1) * ROWS, :] = res.results[c]["out"]
    di = np.arange(N)
    out[:, di, di] = 1.0
    return out


# revision 9
# speedup vs baseline: 1.2793x; 1.0362x over previous
"""Pairwise-distance + global max normalize kernel for trn2, 8 cores.

Problem (hardcoded): x [4, 4096, 64] f32 ->
    out[b] = cdist(x[b], x[b]) / global_max, diag set to 1.0.
    (The reference normalizes (d - dmin)/(dmax - dmin); dmin is the
    diagonal of cdist-via-matmul-identity which rounds to ~0/tiny-neg,
    so dmin = 0: worst-case disagreement < 6e-4 relative, measured
    ~7e-6 end to end.)

Structure per core (SPMD, core c -> batch c//2, row-half c%2):
  - d2 tiles are produced directly by one K=66 matmul: stationary rows
    0:64 = -2*x_rows^T, row 64 = sq_rows, row 65 = ones; moving rows
    0:64 = x_cols^T, row 64 = ones, row 65 = sq_cols.
  - pass A (max scan): only unique pairs are scanned.  Globally the 4
    batches decompose into 40 [1024x1024] quarter-block pairs
    ((q,q) x4 + (q,r) q<r x6 per batch); each core scans 5 of them
    (same shapes on every core -> SPMD-uniform), reduce_max on DVE at
    [128,1024] width from PSUM.
  - AllReduce(max) of the per-partition maxima across the 8 cores.
  - pass B: recompute d2 for this core's [2048,4096] output block,
    out = Sqrt(d2 * (1/max_d2)) on ACT (scale is per-partition SBUF
    operand), DMA to DRAM.  Diagonal d2 can round tiny-negative ->
    Sqrt NaN there; the host overwrites the diagonal with exactly 1.0
    (as the reference does).  Off-diagonal d2 >= ~16 for this data.
"""

import numpy as np

B = 4
N = 4096
D = 64
NCORES = 8
ROWS = N // 2  # 2048 rows per core
K = D + 2  # 66
PT = 128
FT = 512  # one fp32 PSUM bank
WT = 1024  # working tile width (2 banks)
RT = ROWS // PT  # 16 row tiles (pass B)
CG = N // WT  # 4 col groups (pass B)
Q = 1024  # quarter-block size (pass A)
NBLK = 5  # pair-blocks per core
QRT = Q // PT  # 8 row tiles per pair-block

# 40 unique quarter-block pairs (batch, qa, qb); core c takes [5c:5c+5].
PAIR_BLOCKS = [
    (b, qa, qb) for b in range(B) for qa in range(4) for qb in range(qa, 4)
]
assert len(PAIR_BLOCKS) == NCORES * NBLK

_CACHE = {}
LAST_RESULTS = None


def _build_nc():
    import concourse.bacc as bacc
    import concourse.tile as tile
    from concourse import mybir

    f32 = mybir.dt.float32
    nc = bacc.Bacc(None, target_bir_lowering=False)

    kxm = nc.dram_tensor("kxm", [K, ROWS], f32, kind="ExternalInput")
    kxn = nc.dram_tensor("kxn", [K, N], f32, kind="ExternalInput")
    pa = nc.dram_tensor("pa", [K, NBLK * Q], f32, kind="ExternalInput")
    pb = nc.dram_tensor("pb", [K, NBLK * Q], f32, kind="ExternalInput")
    out = nc.dram_tensor("out", [ROWS, N], f32, kind="ExternalOutput")

    with tile.TileContext(nc) as tc:
        with (
            tc.tile_pool(name="singles", bufs=1) as singles,
            tc.tile_pool(name="outp", bufs=4) as outp,
            tc.tile_pool(name="ps", bufs=2, space="PSUM") as psp,
            tc.tile_pool(name="psS", bufs=1, space="PSUM") as psS,
            tc.tile_pool(name="dram", bufs=2, space="DRAM") as dram,
        ):
            pa_s = singles.tile([K, NBLK * Q], f32)
            pb_s = singles.tile([K, NBLK * Q], f32)
            for q in range(NBLK):
                nc.sync.dma_start(out=pa_s[:, q * Q : (q + 1) * Q], in_=pa[:, q * Q : (q + 1) * Q])
                nc.sync.dma_start(out=pb_s[:, q * Q : (q + 1) * Q], in_=pb[:, q * Q : (q + 1) * Q])
            kxm_s = singles.tile([K, ROWS], f32)
            nc.scalar.dma_start(out=kxm_s[:], in_=kxm[:])
            kxn_s = singles.tile([K, N], f32)
            nc.scalar.dma_start(out=kxn_s[:], in_=kxn[:])

            # ---- pass A: max(d2) over this core's 5 unique pair-blocks ----
            stats = singles.tile([PT, NBLK * QRT], f32)
            for q in range(NBLK):
                for rt in range(QRT):
                    ps = psp.tile([PT, WT], f32, tag="ps")
                    for j in range(WT // FT):
                        nc.tensor.matmul(
                            ps[:, j * FT : (j + 1) * FT],
                            pa_s[:, q * Q + rt * PT : q * Q + (rt + 1) * PT],
                            pb_s[:, q * Q + j * FT : q * Q + (j + 1) * FT],
                            start=True,
                            stop=True,
                        )
                    idx = q * QRT + rt
                    nc.vector.reduce_max(
                        out=stats[:, idx : idx + 1],
                        in_=ps[:],
                        axis=mybir.AxisListType.X,
                    )
            loc = singles.tile([PT, 1], f32)
            nc.vector.reduce_max(out=loc[:], in_=stats[:], axis=mybir.AxisListType.X)

            # ---- all-reduce (max) across the 8 cores ----
            inb = dram.tile([1, PT], f32)
            outb = dram.tile([1, PT], f32)
            nc.gpsimd.dma_start(out=inb[:], in_=loc[:])
            nc.gpsimd.collective_compute(
                "AllReduce",
                mybir.AluOpType.max,
                replica_groups=[list(range(NCORES))],
                ins=[inb[:].opt()],
                outs=[outb[:].opt()],
            )
            mxrow = singles.tile([1, PT], f32)
            nc.gpsimd.dma_start(out=mxrow[:], in_=outb[:])
            mx = singles.tile([1, 1], f32)
            nc.vector.reduce_max(out=mx[:], in_=mxrow[:], axis=mybir.AxisListType.X)

            # mx = max(d2) = dmax^2; scale = 1/mx, broadcast via K=1 matmul.
            s2 = singles.tile([1, 1], f32)
            nc.vector.reciprocal(out=s2[:], in_=mx[:])
            ones = singles.tile([1, PT], f32)
            nc.vector.memset(ones[:], 1.0)
            ps_s2 = psS.tile([PT, 1], f32, tag="psS")
            nc.tensor.matmul(ps_s2[:], ones[:], s2[:], start=True, stop=True)
            s2b = singles.tile([PT, 1], f32)
            nc.scalar.copy(out=s2b[:], in_=ps_s2[:])

            # ---- pass B: recompute d2, out = Sqrt(d2/mx), store ----
            for rt in range(RT):
                for cg in range(CG):
                    ps = psp.tile([PT, WT], f32, tag="ps")
                    for j in range(WT // FT):
                        nc.tensor.matmul(
                            ps[:, j * FT : (j + 1) * FT],
                            kxm_s[:, rt * PT : (rt + 1) * PT],
                            kxn_s[:, (cg * 2 + j) * FT : (cg * 2 + j + 1) * FT],
                            start=True,
                            stop=True,
                        )
                    o = outp.tile([PT, WT], f32, tag="o")
                    nc.scalar.activation(
                        out=o[:],
                        in_=ps[:],
                        func=mybir.ActivationFunctionType.Sqrt,
                        bias=0.0,
                        scale=s2b[:],
                    )
                    nc.sync.dma_start(
                        out=out[rt * PT : (rt + 1) * PT, cg * WT : (cg + 1) * WT],
                        in_=o[:],
                    )

    nc.finalize()
    return nc


def _get_nc():
    if "nc" not in _CACHE:
        _CACHE["nc"] = _build_nc()
    return _CACHE["nc"]


def _lhs_block(xblk, sqblk):
    """Stationary-operand layout [K, n]: -2x^T / sq / ones."""
    n = xblk.shape[0]
    m = np.empty((K, n), dtype=np.float32)
    m[:D] = (-2.0 * xblk).T
    m[D] = sqblk
    m[D + 1] = 1.0
    return m


def _rhs_block(xblk, sqblk):
    """Moving-operand layout [K, n]: x^T / ones / sq."""
    n = xblk.shape[0]
    m = np.empty((K, n), dtype=np.float32)
    m[:D] = xblk.T
    m[D] = 1.0
    m[D + 1] = sqblk
    return m


def kernel(x):
    global LAST_RESULTS
    from concourse.bass_utils import run_bass_kernel_spmd

    x = np.asarray(x, dtype=np.float32)
    assert x.shape == (B, N, D), x.shape

    sqs = [(x[b].astype(np.float64) ** 2).sum(-1).astype(np.float32) for b in range(B)]

    in_maps = []
    for c in range(NCORES):
        b, h = divmod(c, 2)
        xb, sq = x[b], sqs[b]
        kxm = _lhs_block(xb[h * ROWS : (h + 1) * ROWS], sq[h * ROWS : (h + 1) * ROWS])
        kxn = _rhs_block(xb, sq)
        pas, pbs = [], []
        for (bb, qa, qb) in PAIR_BLOCKS[c * NBLK : (c + 1) * NBLK]:
            xq, sqq = x[bb], sqs[bb]
            pas.append(_lhs_block(xq[qa * Q : (qa + 1) * Q], sqq[qa * Q : (qa + 1) * Q]))
            pbs.append(_rhs_block(xq[qb * Q : (qb + 1) * Q], sqq[qb * Q : (qb + 1) * Q]))
        pa = np.ascontiguousarray(np.concatenate(pas, axis=1))
        pb = np.ascontiguousarray(np.concatenate(pbs, axis=1))
        in_maps.append(
            {
                "kxm": np.ascontiguousarray(kxm),
                "kxn": np.ascontiguousarray(kxn),
                "pa": pa,
                "pb": pb,
            }
        )

    nc = _get_nc()
    res = run_bass_kernel_spmd(nc, in_maps, core_ids=list(range(NCORES)))
    LAST_RESULTS = res

    out = np.empty((B, N, N), dtype=np.float32)
    for c in range(NCORES):
        b, h = divmod(c, 2)
        out[b, h * ROWS : (h + 1) * ROWS, :] = res.results[c]["out"]
    di = np.arange(N)
    out[:, di, di] = 1.0
    return out
